# revision 11
# baseline (speedup 1.0000x reference)
"""All-pole IIR filter (order 16) on 8 Trainium2 NeuronCores.

Math: y[t] = x[t] - sum_{k=1..16} a_k y[t-k]  (per (b,c) lane, zero init state).

The coefficients are small (0.03*randn tails), so the impulse response h
decays geometrically (spectral radius <~0.91); truncating to 128 taps gives
rel err < 1e-6. Since a[...,0]=1, h[0]=1 exactly, so split

    y = x + c,   c = g * x,   g = h[1:128]   (correction convolution)

and compute ONLY c on device; the host adds back the exact f32 x. All
device streams then carry "small" data (||g||/||h|| ~ 0.12-0.19 per lane),
so fp8e4m3 quantization of x and c contributes only ~3-4e-3 global rel
error (harness gate: 2e-2). Measured rel err ~4.9e-3.

Blocking by Q=128 time steps: c[128c+i] = sum_q W0[q,i] x[128c+q]
+ sum_q W1[q,i] x[128(c-1)+q], with W0/W1 the within/cross-chunk halves of
the Toeplitz operator of g. Because g has <128 taps, W0 and W1 are
complementary triangles and pack into ONE circulant-like matrix per lane:

    Uc[q,j] = g[(127-q-j) mod 128]   (j = 127-i: output flipped, host unflips)

so the weight stream halves to Q*Q fp16 per lane. On device the two
stationaries are unpacked with two affine_selects (keep q+j<=127 -> U0 on
DVE; q+j>=128 -> U1 on GpSimd) -- no extra HBM traffic.

Per-core HBM traffic (32 lanes): x fp8 2 MiB + wc fp16 1 MiB + c fp8 2 MiB
= 5 MiB vs the ~358 GB/s/core DMA limit -> ~15 us stream + ~6 us NEFF
preamble + ~3 us drain/teardown.

PRECISION:
  "corrf8":   x fp8e4, wc fp16, c fp8e4 (5 MiB/core) -- DEFAULT
  "fp16pure": legacy full-filter fp16 x/w/y (10 MiB/core), rel ~2.9e-4
"""

import numpy as np
from contextlib import ExitStack

B, C, T = 32, 8, 65536
L = B * C              # 256 independent lanes
NCORES = 8
LPC = L // NCORES      # 32 lanes per core
Q = 128                # chunk length = contraction dim
NCH = T // Q           # 512 chunks per lane
KTAPS = 128            # truncated FIR length (incl. tap 0)
GRP = 4                # lanes per compute/store group
XGRP = 8               # lanes per x DMA group
WGRP = 8               # lanes per weight DMA chunk / unpack group

PRECISION = "corrf8"

_cache = {}


def _build_corr():
    """Correction-filter kernel: c = g*x with packed circulant weights."""
    import concourse.tile as tile
    from concourse import bacc, mybir

    F32 = mybir.dt.float32
    F16 = mybir.dt.float16
    F8 = mybir.dt.float8e4
    nc = bacc.Bacc("TRN2", target_bir_lowering=False, debug=False)

    # Per-core DRAM layouts (lane-minor so per-partition rows are contiguous):
    #   xq: [Q, LPC, NCH]  x[q, l, c] = x_l[128c + q]          (fp8e4m3)
    #   wc: [Q, LPC, Q]    packed circulant Uc per lane        (fp16)
    #   c:  [Q, LPC, NCH]  c[j, l, ch] = c_l[128ch + 127 - j]  (fp8e4m3)
    xq_d = nc.dram_tensor("xq", [Q, LPC, NCH], F8, kind="ExternalInput")
    wc_d = nc.dram_tensor("wc", [Q, LPC, Q], F16, kind="ExternalInput")
    c_d = nc.dram_tensor("c", [Q, LPC, NCH], F8, kind="ExternalOutput")

    NW = LPC // WGRP
    with tile.TileContext(nc) as tc:
        with ExitStack() as ctx:
            wpool = ctx.enter_context(tc.tile_pool(name="w", bufs=1))
            upool = ctx.enter_context(tc.tile_pool(name="u", bufs=1))
            xpool = ctx.enter_context(tc.tile_pool(name="x", bufs=4))
            ypool = ctx.enter_context(tc.tile_pool(name="y", bufs=6))
            pspool = ctx.enter_context(
                tc.tile_pool(name="ps", bufs=8, space="PSUM")
            )

            # Stream all packed weights on the ACT ring (idle until stores).
            wc_sb = []
            for k in range(NW):
                wt = wpool.tile([Q, WGRP, Q], F16, tag=f"wc{k}", name=f"wc{k}")
                wc_sb.append(wt)
                nc.scalar.dma_start(wt[:], wc_d.ap()[:, k * WGRP : (k + 1) * WGRP, :])

            # Unpack the complementary triangles: iota = 127 - q - j.
            u_sb = []
            for k in range(NW):
                u0 = upool.tile([Q, WGRP, Q], F16, tag=f"u0{k}", name=f"u0{k}")
                u1 = upool.tile([Q, WGRP, Q], F16, tag=f"u1{k}", name=f"u1{k}")
                u_sb.append((u0, u1))
                nc.gpsimd.affine_select(
                    u0[:], wc_sb[k][:],
                    pattern=[[0, WGRP], [-1, Q]],
                    compare_op=mybir.AluOpType.is_ge,
                    fill=0.0, base=127, channel_multiplier=-1,
                )
                nc.vector.tensor_sub(u1[:], wc_sb[k][:], u0[:])

            for gx in range(LPC // XGRP):
                xgsl = slice(gx * XGRP, (gx + 1) * XGRP)
                xh = xpool.tile([Q, XGRP, NCH], F8, tag="xq", name="xq_t")
                if gx == 0:
                    # lane 0 fine-grained so the first matmul's dependency
                    # is tiny; the rest of the group as one big transfer
                    nc.sync.dma_start(xh[:, 0:1, :], xq_d.ap()[:, 0:1, :])
                    nc.sync.dma_start(xh[:, 1:XGRP, :], xq_d.ap()[:, 1:XGRP, :])
                else:
                    nc.sync.dma_start(xh[:], xq_d.ap()[:, xgsl, :])
                for g in range(gx * XGRP // GRP, (gx + 1) * XGRP // GRP):
                    gsl = slice(g * GRP, (g + 1) * GRP)
                    ct = ypool.tile([Q, GRP, NCH], F8, tag="c", name="c_t")
                    for j in range(GRP):
                        lane = g * GRP + j
                        jx = lane - gx * XGRP
                        wk, wl = lane // WGRP, lane % WGRP
                        u0, u1 = u_sb[wk]
                        ps = pspool.tile([Q, NCH], F32, tag="ps", name="ps_t")
                        mm = nc.tensor.matmul
                        mm(ps[:, :], u0[:, wl, :], xh[:, jx, :],
                           start=True, stop=False)
                        mm(ps[:, 1:NCH], u1[:, wl, :], xh[:, jx, 0 : NCH - 1],
                           start=False, stop=True)
                        # PSUM->SBUF fp8; alternate engines so neither ACT
                        # nor DVE exceeds the DMA stream time
                        if j % 2 == 0:
                            nc.scalar.copy(ct[:, j, :], ps[:, :])
                        else:
                            nc.vector.tensor_copy(ct[:, j, :], ps[:, :])
                    if g == LPC // GRP - 1:
                        # halve the final store so the drain tail is shorter
                        h2 = GRP // 2
                        nc.scalar.dma_start(
                            c_d.ap()[:, g * GRP : g * GRP + h2, :], ct[:, 0:h2, :]
                        )
                        nc.scalar.dma_start(
                            c_d.ap()[:, g * GRP + h2 : (g + 1) * GRP, :],
                            ct[:, h2:GRP, :],
                        )
                    else:
                        nc.scalar.dma_start(c_d.ap()[:, gsl, :], ct[:])

    nc.compile()
    return nc


def _build_fp16pure():
    """Legacy full-filter fp16 kernel (x fp16, w fp16, y fp16; 256 taps)."""
    import concourse.tile as tile
    from concourse import bacc, mybir

    F32 = mybir.dt.float32
    F16 = mybir.dt.float16
    nc = bacc.Bacc("TRN2", target_bir_lowering=False, debug=False)

    xh_d = nc.dram_tensor("xh", [Q, LPC, NCH], F16, kind="ExternalInput")
    w_d = {
        n: nc.dram_tensor(n, [Q, LPC, Q], F16, kind="ExternalInput")
        for n in ("w0h", "w1h")
    }
    y_d = nc.dram_tensor("yt", [Q, LPC, NCH], F16, kind="ExternalOutput")

    with tile.TileContext(nc) as tc:
        with ExitStack() as ctx:
            wpool = ctx.enter_context(tc.tile_pool(name="w", bufs=1))
            xpool = ctx.enter_context(tc.tile_pool(name="x", bufs=4))
            ypool = ctx.enter_context(tc.tile_pool(name="y", bufs=6))
            pspool = ctx.enter_context(
                tc.tile_pool(name="ps", bufs=8, space="PSUM")
            )

            wbounds = [0, 1, WGRP] + list(range(2 * WGRP, LPC + 1, WGRP))
            w_sb = {}
            for n in w_d:
                w_sb[n] = [
                    wpool.tile(
                        [Q, wbounds[k + 1] - wbounds[k], Q], F16,
                        tag=f"{n}_{k}", name=f"{n}_{k}",
                    )
                    for k in range(len(wbounds) - 1)
                ]
            for k in range(len(wbounds) - 1):
                sl = slice(wbounds[k], wbounds[k + 1])
                for n in w_d:
                    nc.scalar.dma_start(w_sb[n][k][:], w_d[n].ap()[:, sl, :])

            for gx in range(LPC // XGRP):
                xgsl = slice(gx * XGRP, (gx + 1) * XGRP)
                xh = xpool.tile([Q, XGRP, NCH], F16, tag="xh", name="xh_t")
                if gx == 0:
                    nc.sync.dma_start(xh[:, 0:1, :], xh_d.ap()[:, 0:1, :])
                    nc.sync.dma_start(xh[:, 1:XGRP, :], xh_d.ap()[:, 1:XGRP, :])
                else:
                    nc.sync.dma_start(xh[:], xh_d.ap()[:, xgsl, :])
                for g in range(gx * XGRP // GRP, (gx + 1) * XGRP // GRP):
                    gsl = slice(g * GRP, (g + 1) * GRP)
                    yt = ypool.tile([Q, GRP, NCH], F16, tag="y", name="y_t")
                    for j in range(GRP):
                        lane = g * GRP + j
                        jx = lane - gx * XGRP
                        wk = next(
                            kk for kk in range(len(wbounds) - 1)
                            if lane < wbounds[kk + 1]
                        )
                        wl = lane - wbounds[wk]
                        ps = pspool.tile([Q, NCH], F32, tag="ps", name="ps_t")
                        mm = nc.tensor.matmul
                        mm(ps[:, :], w_sb["w0h"][wk][:, wl, :], xh[:, jx, :],
                           start=True, stop=False)
                        mm(ps[:, 1:NCH], w_sb["w1h"][wk][:, wl, :],
                           xh[:, jx, 0 : NCH - 1], start=False, stop=True)
                        if j % 2 == 0:
                            nc.scalar.copy(yt[:, j, :], ps[:, :])
                        else:
                            nc.vector.tensor_copy(yt[:, j, :], ps[:, :])
                    if g == LPC // GRP - 1:
                        h2 = GRP // 2
                        nc.scalar.dma_start(
                            y_d.ap()[:, g * GRP : g * GRP + h2, :], yt[:, 0:h2, :]
                        )
                        nc.scalar.dma_start(
                            y_d.ap()[:, g * GRP + h2 : (g + 1) * GRP, :],
                            yt[:, h2:GRP, :],
                        )
                    else:
                        nc.scalar.dma_start(y_d.ap()[:, gsl, :], yt[:])

    nc.compile()
    return nc


def _get_bass():
    key = ("nc", PRECISION)
    if key not in _cache:
        _cache[key] = (
            _build_corr() if PRECISION == "corrf8" else _build_fp16pure()
        )
    return _cache[key]


def _impulse_response(a: np.ndarray, ktaps: int) -> np.ndarray:
    """h[l, n] for n in [0, ktaps), float64 recurrence."""
    an = (a.astype(np.float64) / a[..., 0:1].astype(np.float64)).reshape(L, 17)
    h = np.zeros((L, ktaps), np.float64)
    h[:, 0] = 1.0
    for n in range(1, ktaps):
        k = np.arange(1, min(n, 16) + 1)
        h[:, n] = -np.einsum("lk,lk->l", an[:, k], h[:, n - k])
    return h


def _run(in_maps):
    from concourse import bass_utils

    nc = _get_bass()
    res = bass_utils.run_bass_kernel_spmd(
        nc,
        in_maps,
        core_ids=list(range(NCORES)),
        trace=bool(_cache.get("trace", False)),
        trace_cores=_cache.get("trace_cores"),
    )
    _cache["last_results"] = res
    return res


def _kernel_corr(x: np.ndarray) -> np.ndarray:
    import ml_dtypes

    F8 = ml_dtypes.float8_e4m3

    a = _cache.pop("a_pending")
    g = _impulse_response(a, KTAPS).astype(np.float32)
    g[:, 0] = 0.0  # tap 0 handled exactly on host (y = x + c)
    qi = np.arange(Q)
    idx = (127 - np.add.outer(qi, qi)) % 128  # Uc[q,j] = g[(127-q-j) mod 128]
    wc_all = np.ascontiguousarray(g[:, idx])  # [L, q, j] fp32

    xq = np.ascontiguousarray(
        x.reshape(L, NCH, Q).transpose(2, 0, 1)
    ).astype(F8)  # [q, lane, c]
    wc16 = wc_all.astype(np.float16)

    in_maps = []
    for core in range(NCORES):
        sl = slice(core * LPC, (core + 1) * LPC)
        in_maps.append(
            {
                "xq": np.ascontiguousarray(xq[:, sl, :]),
                "wc": np.ascontiguousarray(wc16[sl].transpose(1, 0, 2)),
            }
        )
    res = _run(in_maps)

    y = np.empty((L, T), np.float32)
    for core in range(NCORES):
        c = res.results[core]["c"].astype(np.float32)  # [j, lane, ch]
        sl = slice(core * LPC, (core + 1) * LPC)
        # c[j, l, ch] = c_l[128ch + 127 - j]: unflip j, then chunk-major -> time
        y[sl] = c[::-1].transpose(1, 2, 0).reshape(LPC, T)
    y += x.reshape(L, T)
    return y.reshape(B, C, T)


def _kernel_fp16pure(x: np.ndarray) -> np.ndarray:
    a = _cache.pop("a_pending")
    h = _impulse_response(a, 256).astype(np.float32)  # [L, 256]
    qi = np.arange(Q)
    d = qi[None, :] - qi[:, None]  # d[q, i] = i - q
    w0 = np.where(d >= 0, h[:, np.clip(d, 0, 255)], 0.0).astype(np.float32)
    w1 = h[:, d + Q].astype(np.float32)  # [L, q, i]

    xq = np.ascontiguousarray(
        x.reshape(L, NCH, Q).transpose(2, 0, 1)
    ).astype(np.float16)
    wmats = {"w0h": w0.astype(np.float16), "w1h": w1.astype(np.float16)}

    in_maps = []
    for core in range(NCORES):
        sl = slice(core * LPC, (core + 1) * LPC)
        m = {"xh": np.ascontiguousarray(xq[:, sl, :])}
        for n, w in wmats.items():
            m[n] = np.ascontiguousarray(w[sl].transpose(1, 0, 2))
        in_maps.append(m)
    res = _run(in_maps)

    y = np.empty((L, T), np.float32)
    for core in range(NCORES):
        yt = res.results[core]["yt"].astype(np.float32)  # [i, lane, c]
        sl = slice(core * LPC, (core + 1) * LPC)
        y[sl] = yt.transpose(1, 2, 0).reshape(LPC, T)
    return y.reshape(B, C, T)


def kernel(x: np.ndarray, a: np.ndarray) -> np.ndarray:
    x = np.ascontiguousarray(x, dtype=np.float32)
    a = np.ascontiguousarray(a, dtype=np.float32)
    _cache["a_pending"] = a
    if PRECISION == "corrf8":
        return _kernel_corr(x)
    return _kernel_fp16pure(x)


# revision 20
# speedup vs baseline: 1.1457x; 1.1457x over previous
"""All-pole IIR filter (order 16) on 8 Trainium2 NeuronCores.

Math: y[t] = x[t] - sum_{k=1..16} a_k y[t-k]  (per (b,c) lane, zero init state).

The coefficients are small (0.03*randn tails), so the impulse response h
decays geometrically (spectral radius <~0.91); truncating to 128 taps gives
rel err < 1e-6. Since a[...,0]=1, h[0]=1 exactly, so split

    y = x + c,   c = g * x,   g = h[1:128]   (correction convolution)

and compute ONLY c on device; the host adds back the exact f32 x. All
device streams then carry "small" data (||g||/||h|| ~ 0.12-0.19 per lane),
so fp8e4m3 quantization of x and c contributes only ~3-4e-3 global rel
error (harness gate: 2e-2). Measured rel err ~4.9e-3.

Blocking by Q=128 time steps: c[128c+i] = sum_q W0[q,i] x[128c+q]
+ sum_q W1[q,i] x[128(c-1)+q], with W0/W1 the within/cross-chunk halves of
the Toeplitz operator of g. Because g has <128 taps, W0 and W1 are
complementary triangles and pack into ONE circulant-like matrix per lane:

    Uc[q,j] = g[(127-q-j) mod 128]   (j = 127-i: output flipped, host unflips)

so the weight stream halves to Q*Q fp16 per lane. On device the two
stationaries are unpacked with two affine_selects (keep q+j<=127 -> U0 on
DVE; q+j>=128 -> U1 on GpSimd) -- no extra HBM traffic.

Per-core HBM traffic (32 lanes): x fp8 2 MiB + wc fp16 1 MiB + c fp8 2 MiB
= 5 MiB vs the ~358 GB/s/core DMA limit -> ~15 us stream + ~6 us NEFF
preamble + ~3 us drain/teardown.

PRECISION:
  "corrf8":   x fp8e4, wc fp16, c fp8e4 (5 MiB/core) -- DEFAULT
  "fp16pure": legacy full-filter fp16 x/w/y (10 MiB/core), rel ~2.9e-4
"""

import numpy as np
from contextlib import ExitStack

B, C, T = 32, 8, 65536
L = B * C              # 256 independent lanes
NCORES = 8
LPC = L // NCORES      # 32 lanes per core
Q = 128                # chunk length = contraction dim
NCH = T // Q           # 512 chunks per lane
KTAPS = 128            # truncated FIR length (incl. tap 0)
GRP = 4                # lanes per compute/store group (fp16pure)
XGRP = 16              # lanes per x DMA group
WGRP = 8               # lanes per weight DMA chunk / unpack group

PRECISION = "corrf8"

_cache = {}


def _build_corr():
    """Correction-filter kernel: c = g*x with packed circulant weights.

    All streams fp8e4m3. Per lane two DoublePixel matmuls (fp8 runs 2
    moving cols/cycle; verified bit-identical semantics to the plain
    matmul) accumulate u0^T x_c + u1^T x_{c-1} in one PSUM group. The x
    tile carries a host-written zero column 0 so chunk -1 reads zeros and
    both matmuls stream an even 512 columns.
    """
    import concourse.tile as tile
    from concourse import bacc, mybir

    F32 = mybir.dt.float32
    F8 = mybir.dt.float8e4
    DP = mybir.MatmulPerfMode.DoublePixel
    nc = bacc.Bacc("TRN2", target_bir_lowering=False, debug=False)

    NCH1 = NCH + 1
    # Per-core DRAM layouts (lane-minor so per-partition rows are contiguous):
    #   xq: [Q, LPC, NCH1]  x[q, l, 1+c] = x_l[128c + q], col 0 = zeros
    #   wc: [Q, LPC, Q]     packed circulant Uc per lane
    #   c:  [Q, LPC, NCH]   c[j, l, ch] = c_l[128ch + 127 - j]
    xq_d = nc.dram_tensor("xq", [Q, LPC, NCH1], F8, kind="ExternalInput")
    wc_d = nc.dram_tensor("wc", [Q, LPC, Q], F8, kind="ExternalInput")
    c_d = nc.dram_tensor("c", [Q, LPC, NCH], F8, kind="ExternalOutput")

    NW = LPC // WGRP
    with tile.TileContext(nc) as tc:
        with ExitStack() as ctx:
            wpool = ctx.enter_context(tc.tile_pool(name="w", bufs=1))
            upool = ctx.enter_context(tc.tile_pool(name="u", bufs=1))
            xpool = ctx.enter_context(tc.tile_pool(name="x", bufs=2))
            ypool = ctx.enter_context(tc.tile_pool(name="y", bufs=3))
            pspool = ctx.enter_context(
                tc.tile_pool(name="ps", bufs=8, space="PSUM")
            )

            # Packed weights on the GPSIMD ring (otherwise idle at start;
            # the unpack that consumes them lives there too).
            wc_sb = []
            for k in range(NW):
                wt = wpool.tile([Q, WGRP, Q], F8, tag=f"wc{k}", name=f"wc{k}")
                wc_sb.append(wt)
                nc.gpsimd.dma_start(wt[:], wc_d.ap()[:, k * WGRP : (k + 1) * WGRP, :])

            # Unpack complementary triangles: slot 0 = u0 (keep q+j<=127,
            # iota = 127-q-j >= 0), slot 1 = u1 = wc - u0.
            # Group 0 split in halves so lane 0's stationary is ready sooner.
            u_sb = []
            for k in range(NW):
                u01 = upool.tile(
                    [Q, WGRP, 2, Q], F8, tag=f"u{k}", name=f"u{k}"
                )
                u_sb.append(u01)
                halves = (
                    [(0, WGRP // 2), (WGRP // 2, WGRP)] if k == 0
                    else [(0, WGRP)]
                )
                for lo, hi in halves:
                    nc.gpsimd.affine_select(
                        u01[:, lo:hi, 0, :], wc_sb[k][:, lo:hi, :],
                        pattern=[[0, hi - lo], [-1, Q]],
                        compare_op=mybir.AluOpType.is_ge,
                        fill=0.0, base=127, channel_multiplier=-1,
                    )
                    nc.gpsimd.tensor_sub(
                        u01[:, lo:hi, 1, :], wc_sb[k][:, lo:hi, :],
                        u01[:, lo:hi, 0, :],
                    )

            SGRP = 8  # lanes per store group
            for gx in range(LPC // XGRP):
                xgsl = slice(gx * XGRP, (gx + 1) * XGRP)
                xt = xpool.tile([Q, XGRP, NCH1], F8, tag="xq", name="xq_t")
                if gx == 0:
                    # lane 0 fine-grained so the first matmul's dependency
                    # is tiny; the rest of the group as one big transfer
                    nc.sync.dma_start(xt[:, 0:1, :], xq_d.ap()[:, 0:1, :])
                    nc.sync.dma_start(
                        xt[:, 1:XGRP, :], xq_d.ap()[:, 1:XGRP, :]
                    )
                else:
                    nc.sync.dma_start(xt[:], xq_d.ap()[:, xgsl, :])
                for g in range(gx * XGRP // SGRP, (gx + 1) * XGRP // SGRP):
                    gsl = slice(g * SGRP, (g + 1) * SGRP)
                    ct = ypool.tile([Q, SGRP, NCH], F8, tag="c", name="c_t")
                    for j in range(SGRP):
                        lane = g * SGRP + j
                        jx = lane - gx * XGRP
                        wk, wl = lane // WGRP, lane % WGRP
                        u01 = u_sb[wk]
                        ps = pspool.tile([Q, NCH], F32, tag="ps", name="ps_t")
                        nc.tensor.matmul(
                            ps[:, :], u01[:, wl, 0, :], xt[:, jx, 1:NCH1],
                            start=True, stop=False, perf_mode=DP,
                        )
                        nc.tensor.matmul(
                            ps[:, :], u01[:, wl, 1, :], xt[:, jx, 0:NCH],
                            start=False, stop=True, perf_mode=DP,
                        )
                        # PSUM->SBUF fp8 casts alternate ACT/DVE (GPSIMD
                        # cannot read PSUM) so neither engine exceeds the
                        # DMA stream time
                        if j % 2 == 0:
                            nc.scalar.copy(ct[:, j, :], ps[:, :])
                        else:
                            nc.vector.tensor_copy(ct[:, j, :], ps[:, :])
                    if g == LPC // SGRP - 1:
                        # halve the final store so the drain tail is shorter
                        h2 = SGRP // 2
                        nc.sync.dma_start(
                            c_d.ap()[:, g * SGRP : g * SGRP + h2, :],
                            ct[:, 0:h2, :],
                        )
                        nc.sync.dma_start(
                            c_d.ap()[:, g * SGRP + h2 : (g + 1) * SGRP, :],
                            ct[:, h2:SGRP, :],
                        )
                    else:
                        nc.sync.dma_start(c_d.ap()[:, gsl, :], ct[:])

    nc.compile()
    return nc


def _build_fp16pure():
    """Legacy full-filter fp16 kernel (x fp16, w fp16, y fp16; 256 taps)."""
    import concourse.tile as tile
    from concourse import bacc, mybir

    F32 = mybir.dt.float32
    F16 = mybir.dt.float16
    nc = bacc.Bacc("TRN2", target_bir_lowering=False, debug=False)

    xh_d = nc.dram_tensor("xh", [Q, LPC, NCH], F16, kind="ExternalInput")
    w_d = {
        n: nc.dram_tensor(n, [Q, LPC, Q], F16, kind="ExternalInput")
        for n in ("w0h", "w1h")
    }
    y_d = nc.dram_tensor("yt", [Q, LPC, NCH], F16, kind="ExternalOutput")

    with tile.TileContext(nc) as tc:
        with ExitStack() as ctx:
            wpool = ctx.enter_context(tc.tile_pool(name="w", bufs=1))
            xpool = ctx.enter_context(tc.tile_pool(name="x", bufs=4))
            ypool = ctx.enter_context(tc.tile_pool(name="y", bufs=6))
            pspool = ctx.enter_context(
                tc.tile_pool(name="ps", bufs=8, space="PSUM")
            )

            wbounds = [0, 1, WGRP] + list(range(2 * WGRP, LPC + 1, WGRP))
            w_sb = {}
            for n in w_d:
                w_sb[n] = [
                    wpool.tile(
                        [Q, wbounds[k + 1] - wbounds[k], Q], F16,
                        tag=f"{n}_{k}", name=f"{n}_{k}",
                    )
                    for k in range(len(wbounds) - 1)
                ]
            for k in range(len(wbounds) - 1):
                sl = slice(wbounds[k], wbounds[k + 1])
                for n in w_d:
                    nc.scalar.dma_start(w_sb[n][k][:], w_d[n].ap()[:, sl, :])

            for gx in range(LPC // XGRP):
                xgsl = slice(gx * XGRP, (gx + 1) * XGRP)
                xh = xpool.tile([Q, XGRP, NCH], F16, tag="xh", name="xh_t")
                if gx == 0:
                    nc.sync.dma_start(xh[:, 0:1, :], xh_d.ap()[:, 0:1, :])
                    nc.sync.dma_start(xh[:, 1:XGRP, :], xh_d.ap()[:, 1:XGRP, :])
                else:
                    nc.sync.dma_start(xh[:], xh_d.ap()[:, xgsl, :])
                for g in range(gx * XGRP // GRP, (gx + 1) * XGRP // GRP):
                    gsl = slice(g * GRP, (g + 1) * GRP)
                    yt = ypool.tile([Q, GRP, NCH], F16, tag="y", name="y_t")
                    for j in range(GRP):
                        lane = g * GRP + j
                        jx = lane - gx * XGRP
                        wk = next(
                            kk for kk in range(len(wbounds) - 1)
                            if lane < wbounds[kk + 1]
                        )
                        wl = lane - wbounds[wk]
                        ps = pspool.tile([Q, NCH], F32, tag="ps", name="ps_t")
                        mm = nc.tensor.matmul
                        mm(ps[:, :], w_sb["w0h"][wk][:, wl, :], xh[:, jx, :],
                           start=True, stop=False)
                        mm(ps[:, 1:NCH], w_sb["w1h"][wk][:, wl, :],
                           xh[:, jx, 0 : NCH - 1], start=False, stop=True)
                        if j % 2 == 0:
                            nc.scalar.copy(yt[:, j, :], ps[:, :])
                        else:
                            nc.vector.tensor_copy(yt[:, j, :], ps[:, :])
                    if g == LPC // GRP - 1:
                        h2 = GRP // 2
                        nc.scalar.dma_start(
                            y_d.ap()[:, g * GRP : g * GRP + h2, :], yt[:, 0:h2, :]
                        )
                        nc.scalar.dma_start(
                            y_d.ap()[:, g * GRP + h2 : (g + 1) * GRP, :],
                            yt[:, h2:GRP, :],
                        )
                    else:
                        nc.scalar.dma_start(y_d.ap()[:, gsl, :], yt[:])

    nc.compile()
    return nc


def _get_bass():
    key = ("nc", PRECISION)
    if key not in _cache:
        _cache[key] = (
            _build_corr() if PRECISION == "corrf8" else _build_fp16pure()
        )
    return _cache[key]


def _impulse_response(a: np.ndarray, ktaps: int) -> np.ndarray:
    """h[l, n] for n in [0, ktaps), float64 recurrence."""
    an = (a.astype(np.float64) / a[..., 0:1].astype(np.float64)).reshape(L, 17)
    h = np.zeros((L, ktaps), np.float64)
    h[:, 0] = 1.0
    for n in range(1, ktaps):
        k = np.arange(1, min(n, 16) + 1)
        h[:, n] = -np.einsum("lk,lk->l", an[:, k], h[:, n - k])
    return h


def _run(in_maps):
    from concourse import bass_utils

    nc = _get_bass()
    res = bass_utils.run_bass_kernel_spmd(
        nc,
        in_maps,
        core_ids=list(range(NCORES)),
        trace=bool(_cache.get("trace", False)),
        trace_cores=_cache.get("trace_cores"),
    )
    _cache["last_results"] = res
    return res


def _kernel_corr(x: np.ndarray) -> np.ndarray:
    import ml_dtypes

    F8 = ml_dtypes.float8_e4m3

    a = _cache.pop("a_pending")
    g = _impulse_response(a, KTAPS).astype(np.float32)
    g[:, 0] = 0.0  # tap 0 handled exactly on host (y = x + c)
    qi = np.arange(Q)
    idx = (127 - np.add.outer(qi, qi)) % 128  # Uc[q,j] = g[(127-q-j) mod 128]
    wc_all = np.ascontiguousarray(g[:, idx])  # [L, q, j] fp32

    xq = np.zeros((Q, L, NCH + 1), F8)  # col 0 stays zero (chunk -1)
    xq[:, :, 1:] = x.reshape(L, NCH, Q).transpose(2, 0, 1).astype(F8)
    wc8 = wc_all.astype(F8)

    in_maps = []
    for core in range(NCORES):
        sl = slice(core * LPC, (core + 1) * LPC)
        in_maps.append(
            {
                "xq": np.ascontiguousarray(xq[:, sl, :]),
                "wc": np.ascontiguousarray(wc8[sl].transpose(1, 0, 2)),
            }
        )
    res = _run(in_maps)

    y = np.empty((L, T), np.float32)
    for core in range(NCORES):
        c = res.results[core]["c"].astype(np.float32)  # [j, lane, ch]
        sl = slice(core * LPC, (core + 1) * LPC)
        # c[j, l, ch] = c_l[128ch + 127 - j]: unflip j, then chunk-major -> time
        y[sl] = c[::-1].transpose(1, 2, 0).reshape(LPC, T)
    y += x.reshape(L, T)
    return y.reshape(B, C, T)


def _kernel_fp16pure(x: np.ndarray) -> np.ndarray:
    a = _cache.pop("a_pending")
    h = _impulse_response(a, 256).astype(np.float32)  # [L, 256]
    qi = np.arange(Q)
    d = qi[None, :] - qi[:, None]  # d[q, i] = i - q
    w0 = np.where(d >= 0, h[:, np.clip(d, 0, 255)], 0.0).astype(np.float32)
    w1 = h[:, d + Q].astype(np.float32)  # [L, q, i]

    xq = np.ascontiguousarray(
        x.reshape(L, NCH, Q).transpose(2, 0, 1)
    ).astype(np.float16)
    wmats = {"w0h": w0.astype(np.float16), "w1h": w1.astype(np.float16)}

    in_maps = []
    for core in range(NCORES):
        sl = slice(core * LPC, (core + 1) * LPC)
        m = {"xh": np.ascontiguousarray(xq[:, sl, :])}
        for n, w in wmats.items():
            m[n] = np.ascontiguousarray(w[sl].transpose(1, 0, 2))
        in_maps.append(m)
    res = _run(in_maps)

    y = np.empty((L, T), np.float32)
    for core in range(NCORES):
        yt = res.results[core]["yt"].astype(np.float32)  # [i, lane, c]
        sl = slice(core * LPC, (core + 1) * LPC)
        y[sl] = yt.transpose(1, 2, 0).reshape(LPC, T)
    return y.reshape(B, C, T)


def kernel(x: np.ndarray, a: np.ndarray) -> np.ndarray:
    x = np.ascontiguousarray(x, dtype=np.float32)
    a = np.ascontiguousarray(a, dtype=np.float32)
    _cache["a_pending"] = a
    if PRECISION == "corrf8":
        return _kernel_corr(x)
    return _kernel_fp16pure(x)


# revision 23
# speedup vs baseline: 1.1680x; 1.0195x over previous
"""All-pole IIR filter (order 16) on 8 Trainium2 NeuronCores.

Math: y[t] = x[t] - sum_{k=1..16} a_k y[t-k]  (per (b,c) lane, zero init state).

The coefficients are small (0.03*randn tails), so the impulse response h
decays geometrically (spectral radius <~0.91); truncating to 128 taps gives
rel err < 1e-6. Since a[...,0]=1, h[0]=1 exactly, so split

    y = x + c,   c = g * x,   g = h[1:128]   (correction convolution)

and compute ONLY c on device; the host adds back the exact f32 x. All
device streams then carry "small" data (||g||/||h|| ~ 0.12-0.19 per lane),
so fp8e4m3 quantization of x and c contributes only ~3-4e-3 global rel
error (harness gate: 2e-2). Measured rel err ~4.9e-3.

Blocking by Q=128 time steps: c[128c+i] = sum_q W0[q,i] x[128c+q]
+ sum_q W1[q,i] x[128(c-1)+q], with W0/W1 the within/cross-chunk halves of
the Toeplitz operator of g. Because g has <128 taps, W0 and W1 are
complementary triangles and pack into ONE circulant-like matrix per lane:

    Uc[q,j] = g[(127-q-j) mod 128]   (j = 127-i: output flipped, host unflips)

so the weight stream halves to Q*Q fp16 per lane. On device the two
stationaries are unpacked with two affine_selects (keep q+j<=127 -> U0 on
DVE; q+j>=128 -> U1 on GpSimd) -- no extra HBM traffic.

Per-core HBM traffic (32 lanes): x fp8 2 MiB + wc fp16 1 MiB + c fp8 2 MiB
= 5 MiB vs the ~358 GB/s/core DMA limit -> ~15 us stream + ~6 us NEFF
preamble + ~3 us drain/teardown.

PRECISION:
  "corrf8":   x fp8e4, wc fp16, c fp8e4 (5 MiB/core) -- DEFAULT
  "fp16pure": legacy full-filter fp16 x/w/y (10 MiB/core), rel ~2.9e-4
"""

import numpy as np
from contextlib import ExitStack

B, C, T = 32, 8, 65536
L = B * C              # 256 independent lanes
NCORES = 8
LPC = L // NCORES      # 32 lanes per core
Q = 128                # chunk length = contraction dim
NCH = T // Q           # 512 chunks per lane
KTAPS = 128            # truncated FIR length (incl. tap 0)
GRP = 4                # lanes per compute/store group (fp16pure)
XGRP = 16              # lanes per x DMA group
WGRP = 8               # lanes per weight DMA chunk / unpack group

PRECISION = "corrf8"

_cache = {}


def _build_corr():
    """Correction-filter kernel: c = g*x with packed circulant weights.

    All streams fp8e4m3. Per lane two DoubleRow matmuls (contraction 256
    = 2 k-tiles of 128, true 2x fp8 rate). HW requires the two k-tile
    blocks of the moving operand to be exactly adjacent (stride == block
    width), so x is laid out per lane as 768 chunk-columns

        [O_0..O_255 | E_0..E_255 | 0 O_0..O_254]

    (E/O = even/odd 128-chunks; odd chunks stored twice). The odd-output
    matmul reads cols [0,512) = (O_m cur, E_m prev), the even-output
    matmul reads cols [256,768) = (E_m cur, ZO_m prev); both use the SAME
    stationary [u0, u1]. Even/odd results land in separate PSUM banks
    (start_tensor_calc zeroes a whole 2 KiB bank) and one strided copy
    per lane casts both to fp8.
    """
    import concourse.tile as tile
    from concourse import bacc, mybir
    from concourse.ap import AP

    F32 = mybir.dt.float32
    F8 = mybir.dt.float8e4
    DR = mybir.MatmulPerfMode.DoubleRow
    nc = bacc.Bacc("TRN2", target_bir_lowering=False, debug=False)

    NM = NCH // 2  # double-chunks (moving columns per matmul)
    XC = 3 * NM    # x columns per lane
    # Per-core DRAM layouts (lane-minor so per-partition rows are contiguous):
    #   xq: [Q, LPC, XC]      x chunk columns [O | E | 0,O_0..254]
    #   wc: [Q, LPC, Q]       packed circulant Uc per lane
    #   c:  [Q, LPC, 2, NM]   c[j, l, e, m] = c_l[128*(2m+e) + 127 - j]
    xq_d = nc.dram_tensor("xq", [Q, LPC, XC], F8, kind="ExternalInput")
    wc_d = nc.dram_tensor("wc", [Q, LPC, Q], F8, kind="ExternalInput")
    c_d = nc.dram_tensor("c", [Q, LPC, 2, NM], F8, kind="ExternalOutput")

    NW = LPC // WGRP
    with tile.TileContext(nc) as tc:
        with ExitStack() as ctx:
            wpool = ctx.enter_context(tc.tile_pool(name="w", bufs=1))
            upool = ctx.enter_context(tc.tile_pool(name="u", bufs=1))
            xpool = ctx.enter_context(tc.tile_pool(name="x", bufs=2))
            ypool = ctx.enter_context(tc.tile_pool(name="y", bufs=3))
            pspool = ctx.enter_context(
                tc.tile_pool(name="ps", bufs=4, space="PSUM")
            )

            # wc group 0 first on the SP ring (small, unblocks the unpack);
            # remaining groups on the ACT ring, idle until copies start.
            wc_sb = []
            for k in range(NW):
                wt = wpool.tile([Q, WGRP, Q], F8, tag=f"wc{k}", name=f"wc{k}")
                wc_sb.append(wt)
                eng = nc.sync if k == 0 else nc.scalar
                eng.dma_start(wt[:], wc_d.ap()[:, k * WGRP : (k + 1) * WGRP, :])

            # Unpack complementary triangles: slot 0 = u0 (keep q+j<=127,
            # iota = 127-q-j >= 0) on GPSIMD, slot 1 = u1 = wc - u0 on DVE.
            # Group 0 split in halves so lane 0's stationary is ready sooner.
            u_sb = []
            for k in range(NW):
                u01 = upool.tile(
                    [Q, WGRP, 2, Q], F8, tag=f"u{k}", name=f"u{k}"
                )
                u_sb.append(u01)
                halves = (
                    [(0, WGRP // 2), (WGRP // 2, WGRP)] if k == 0
                    else [(0, WGRP)]
                )
                for lo, hi in halves:
                    nc.gpsimd.affine_select(
                        u01[:, lo:hi, 0, :], wc_sb[k][:, lo:hi, :],
                        pattern=[[0, hi - lo], [-1, Q]],
                        compare_op=mybir.AluOpType.is_ge,
                        fill=0.0, base=127, channel_multiplier=-1,
                    )
                    nc.gpsimd.tensor_sub(
                        u01[:, lo:hi, 1, :], wc_sb[k][:, lo:hi, :],
                        u01[:, lo:hi, 0, :],
                    )

            SGRP = 8  # lanes per store group
            for gx in range(LPC // XGRP):
                xgsl = slice(gx * XGRP, (gx + 1) * XGRP)
                xt = xpool.tile([Q, XGRP, XC], F8, tag="xq", name="xq_t")
                if gx == 0:
                    # lane 0 fine-grained so the first matmul's dependency
                    # is tiny; the rest of the group as one big transfer
                    nc.sync.dma_start(xt[:, 0:1, :], xq_d.ap()[:, 0:1, :])
                    nc.sync.dma_start(
                        xt[:, 1:XGRP, :], xq_d.ap()[:, 1:XGRP, :]
                    )
                else:
                    nc.sync.dma_start(xt[:], xq_d.ap()[:, xgsl, :])
                for g in range(gx * XGRP // SGRP, (gx + 1) * XGRP // SGRP):
                    gsl = slice(g * SGRP, (g + 1) * SGRP)
                    ct = ypool.tile([Q, SGRP, 2, NM], F8, tag="c", name="c_t")
                    for j in range(SGRP):
                        lane = g * SGRP + j
                        jx = lane - gx * XGRP
                        wk, wl = lane // WGRP, lane % WGRP
                        u01 = u_sb[wk]
                        ps = pspool.tile([Q, 2, NCH], F32, tag="ps", name="ps_t")
                        base = xt[:, jx, :]
                        pstr = list(base.ap[0])
                        xodd = AP(base.tensor, base.offset,
                                  [pstr, [NM, 2], [1, NM]])
                        xeven = AP(base.tensor, base.offset + NM,
                                   [pstr, [NM, 2], [1, NM]])
                        nc.tensor.matmul(
                            ps[:, 1, 0:NM], u01[:, wl, :, :], xodd,
                            start=True, stop=True, perf_mode=DR,
                        )
                        nc.tensor.matmul(
                            ps[:, 0, 0:NM], u01[:, wl, :, :], xeven,
                            start=True, stop=True, perf_mode=DR,
                        )
                        # PSUM->SBUF fp8 casts alternate ACT/DVE (GPSIMD
                        # cannot read PSUM) so neither engine exceeds the
                        # DMA stream time
                        if j % 2 == 0:
                            nc.scalar.copy(ct[:, j, :, :], ps[:, :, 0:NM])
                        else:
                            nc.vector.tensor_copy(ct[:, j, :, :], ps[:, :, 0:NM])
                    if g == LPC // SGRP - 1:
                        # halve the final store so the drain tail is shorter
                        h2 = SGRP // 2
                        nc.sync.dma_start(
                            c_d.ap()[:, g * SGRP : g * SGRP + h2, :, :],
                            ct[:, 0:h2, :, :],
                        )
                        nc.sync.dma_start(
                            c_d.ap()[:, g * SGRP + h2 : (g + 1) * SGRP, :, :],
                            ct[:, h2:SGRP, :, :],
                        )
                    else:
                        nc.sync.dma_start(c_d.ap()[:, gsl, :, :], ct[:])

    nc.compile()
    return nc


def _build_fp16pure():
    """Legacy full-filter fp16 kernel (x fp16, w fp16, y fp16; 256 taps)."""
    import concourse.tile as tile
    from concourse import bacc, mybir

    F32 = mybir.dt.float32
    F16 = mybir.dt.float16
    nc = bacc.Bacc("TRN2", target_bir_lowering=False, debug=False)

    xh_d = nc.dram_tensor("xh", [Q, LPC, NCH], F16, kind="ExternalInput")
    w_d = {
        n: nc.dram_tensor(n, [Q, LPC, Q], F16, kind="ExternalInput")
        for n in ("w0h", "w1h")
    }
    y_d = nc.dram_tensor("yt", [Q, LPC, NCH], F16, kind="ExternalOutput")

    with tile.TileContext(nc) as tc:
        with ExitStack() as ctx:
            wpool = ctx.enter_context(tc.tile_pool(name="w", bufs=1))
            xpool = ctx.enter_context(tc.tile_pool(name="x", bufs=4))
            ypool = ctx.enter_context(tc.tile_pool(name="y", bufs=6))
            pspool = ctx.enter_context(
                tc.tile_pool(name="ps", bufs=8, space="PSUM")
            )

            wbounds = [0, 1, WGRP] + list(range(2 * WGRP, LPC + 1, WGRP))
            w_sb = {}
            for n in w_d:
                w_sb[n] = [
                    wpool.tile(
                        [Q, wbounds[k + 1] - wbounds[k], Q], F16,
                        tag=f"{n}_{k}", name=f"{n}_{k}",
                    )
                    for k in range(len(wbounds) - 1)
                ]
            for k in range(len(wbounds) - 1):
                sl = slice(wbounds[k], wbounds[k + 1])
                for n in w_d:
                    nc.scalar.dma_start(w_sb[n][k][:], w_d[n].ap()[:, sl, :])

            for gx in range(LPC // XGRP):
                xgsl = slice(gx * XGRP, (gx + 1) * XGRP)
                xh = xpool.tile([Q, XGRP, NCH], F16, tag="xh", name="xh_t")
                if gx == 0:
                    nc.sync.dma_start(xh[:, 0:1, :], xh_d.ap()[:, 0:1, :])
                    nc.sync.dma_start(xh[:, 1:XGRP, :], xh_d.ap()[:, 1:XGRP, :])
                else:
                    nc.sync.dma_start(xh[:], xh_d.ap()[:, xgsl, :])
                for g in range(gx * XGRP // GRP, (gx + 1) * XGRP // GRP):
                    gsl = slice(g * GRP, (g + 1) * GRP)
                    yt = ypool.tile([Q, GRP, NCH], F16, tag="y", name="y_t")
                    for j in range(GRP):
                        lane = g * GRP + j
                        jx = lane - gx * XGRP
                        wk = next(
                            kk for kk in range(len(wbounds) - 1)
                            if lane < wbounds[kk + 1]
                        )
                        wl = lane - wbounds[wk]
                        ps = pspool.tile([Q, NCH], F32, tag="ps", name="ps_t")
                        mm = nc.tensor.matmul
                        mm(ps[:, :], w_sb["w0h"][wk][:, wl, :], xh[:, jx, :],
                           start=True, stop=False)
                        mm(ps[:, 1:NCH], w_sb["w1h"][wk][:, wl, :],
                           xh[:, jx, 0 : NCH - 1], start=False, stop=True)
                        if j % 2 == 0:
                            nc.scalar.copy(yt[:, j, :], ps[:, :])
                        else:
                            nc.vector.tensor_copy(yt[:, j, :], ps[:, :])
                    if g == LPC // GRP - 1:
                        h2 = GRP // 2
                        nc.scalar.dma_start(
                            y_d.ap()[:, g * GRP : g * GRP + h2, :], yt[:, 0:h2, :]
                        )
                        nc.scalar.dma_start(
                            y_d.ap()[:, g * GRP + h2 : (g + 1) * GRP, :],
                            yt[:, h2:GRP, :],
                        )
                    else:
                        nc.scalar.dma_start(y_d.ap()[:, gsl, :], yt[:])

    nc.compile()
    return nc


def _get_bass():
    key = ("nc", PRECISION)
    if key not in _cache:
        _cache[key] = (
            _build_corr() if PRECISION == "corrf8" else _build_fp16pure()
        )
    return _cache[key]


def _impulse_response(a: np.ndarray, ktaps: int) -> np.ndarray:
    """h[l, n] for n in [0, ktaps), float64 recurrence."""
    an = (a.astype(np.float64) / a[..., 0:1].astype(np.float64)).reshape(L, 17)
    h = np.zeros((L, ktaps), np.float64)
    h[:, 0] = 1.0
    for n in range(1, ktaps):
        k = np.arange(1, min(n, 16) + 1)
        h[:, n] = -np.einsum("lk,lk->l", an[:, k], h[:, n - k])
    return h


def _run(in_maps):
    from concourse import bass_utils

    nc = _get_bass()
    res = bass_utils.run_bass_kernel_spmd(
        nc,
        in_maps,
        core_ids=list(range(NCORES)),
        trace=bool(_cache.get("trace", False)),
        trace_cores=_cache.get("trace_cores"),
    )
    _cache["last_results"] = res
    return res


def _kernel_corr(x: np.ndarray) -> np.ndarray:
    import ml_dtypes

    F8 = ml_dtypes.float8_e4m3

    a = _cache.pop("a_pending")
    g = _impulse_response(a, KTAPS).astype(np.float32)
    g[:, 0] = 0.0  # tap 0 handled exactly on host (y = x + c)
    qi = np.arange(Q)
    idx = (127 - np.add.outer(qi, qi)) % 128  # Uc[q,j] = g[(127-q-j) mod 128]
    wc_all = np.ascontiguousarray(g[:, idx])  # [L, q, j] fp32

    NM = NCH // 2
    XC = 3 * NM
    x3 = x.reshape(L, NCH, Q)
    E = x3[:, 0::2, :].transpose(2, 0, 1).astype(F8)  # [q, l, m]
    O = x3[:, 1::2, :].transpose(2, 0, 1).astype(F8)
    xq = np.zeros((Q, L, XC), F8)  # col 2*NM stays zero (chunk -1)
    xq[:, :, 0:NM] = O
    xq[:, :, NM : 2 * NM] = E
    xq[:, :, 2 * NM + 1 :] = O[:, :, 0 : NM - 1]
    wc8 = wc_all.astype(F8)

    in_maps = []
    for core in range(NCORES):
        sl = slice(core * LPC, (core + 1) * LPC)
        in_maps.append(
            {
                "xq": np.ascontiguousarray(xq[:, sl, :]),
                "wc": np.ascontiguousarray(wc8[sl].transpose(1, 0, 2)),
            }
        )
    res = _run(in_maps)

    y = np.empty((L, T), np.float32)
    for core in range(NCORES):
        c = res.results[core]["c"].astype(np.float32)  # [j, lane, e, m]
        sl = slice(core * LPC, (core + 1) * LPC)
        # c[j, l, e, m] = c_l[128*(2m+e) + 127 - j]: unflip j, reinterleave
        y[sl] = c[::-1].transpose(1, 3, 2, 0).reshape(LPC, T)
    y += x.reshape(L, T)
    return y.reshape(B, C, T)


def _kernel_fp16pure(x: np.ndarray) -> np.ndarray:
    a = _cache.pop("a_pending")
    h = _impulse_response(a, 256).astype(np.float32)  # [L, 256]
    qi = np.arange(Q)
    d = qi[None, :] - qi[:, None]  # d[q, i] = i - q
    w0 = np.where(d >= 0, h[:, np.clip(d, 0, 255)], 0.0).astype(np.float32)
    w1 = h[:, d + Q].astype(np.float32)  # [L, q, i]

    xq = np.ascontiguousarray(
        x.reshape(L, NCH, Q).transpose(2, 0, 1)
    ).astype(np.float16)
    wmats = {"w0h": w0.astype(np.float16), "w1h": w1.astype(np.float16)}

    in_maps = []
    for core in range(NCORES):
        sl = slice(core * LPC, (core + 1) * LPC)
        m = {"xh": np.ascontiguousarray(xq[:, sl, :])}
        for n, w in wmats.items():
            m[n] = np.ascontiguousarray(w[sl].transpose(1, 0, 2))
        in_maps.append(m)
    res = _run(in_maps)

    y = np.empty((L, T), np.float32)
    for core in range(NCORES):
        yt = res.results[core]["yt"].astype(np.float32)  # [i, lane, c]
        sl = slice(core * LPC, (core + 1) * LPC)
        y[sl] = yt.transpose(1, 2, 0).reshape(LPC, T)
    return y.reshape(B, C, T)


def kernel(x: np.ndarray, a: np.ndarray) -> np.ndarray:
    x = np.ascontiguousarray(x, dtype=np.float32)
    a = np.ascontiguousarray(a, dtype=np.float32)
    _cache["a_pending"] = a
    if PRECISION == "corrf8":
        return _kernel_corr(x)
    return _kernel_fp16pure(x)


# revision 25
# speedup vs baseline: 1.1978x; 1.0255x over previous
"""All-pole IIR filter (order 16) on 8 Trainium2 NeuronCores.

Math: y[t] = x[t] - sum_{k=1..16} a_k y[t-k]  (per (b,c) lane, zero init state).

The coefficients are small (0.03*randn tails), so the impulse response h
decays geometrically (spectral radius <~0.91); truncating to 128 taps gives
rel err < 1e-6. Since a[...,0]=1, h[0]=1 exactly, so split

    y = x + c,   c = g * x,   g = h[1:128]   (correction convolution)

and compute ONLY c on device; the host adds back the exact f32 x. All
device streams then carry "small" data (||g||/||h|| ~ 0.12-0.19 per lane),
so fp8e4m3 quantization of x and c contributes only ~3-4e-3 global rel
error (harness gate: 2e-2). Measured rel err ~4.9e-3.

Blocking by Q=128 time steps: c[128c+i] = sum_q W0[q,i] x[128c+q]
+ sum_q W1[q,i] x[128(c-1)+q], with W0/W1 the within/cross-chunk halves of
the Toeplitz operator of g. Because g has <128 taps, W0 and W1 are
complementary triangles and pack into ONE circulant-like matrix per lane:

    Uc[q,j] = g[(127-q-j) mod 128]   (j = 127-i: output flipped, host unflips)

so the weight stream halves to Q*Q fp16 per lane. On device the two
stationaries are unpacked with two affine_selects (keep q+j<=127 -> U0 on
DVE; q+j>=128 -> U1 on GpSimd) -- no extra HBM traffic.

Per-core HBM traffic (32 lanes): x fp8 2 MiB + wc fp16 1 MiB + c fp8 2 MiB
= 5 MiB vs the ~358 GB/s/core DMA limit -> ~15 us stream + ~6 us NEFF
preamble + ~3 us drain/teardown.

PRECISION:
  "corrf8":   x fp8e4, wc fp16, c fp8e4 (5 MiB/core) -- DEFAULT
  "fp16pure": legacy full-filter fp16 x/w/y (10 MiB/core), rel ~2.9e-4
"""

import numpy as np
from contextlib import ExitStack

B, C, T = 32, 8, 65536
L = B * C              # 256 independent lanes
NCORES = 8
LPC = L // NCORES      # 32 lanes per core
Q = 128                # chunk length = contraction dim
NCH = T // Q           # 512 chunks per lane
KTAPS = 128            # truncated FIR length (incl. tap 0)
GRP = 4                # lanes per compute/store group (fp16pure)
XGRP = 16              # lanes per x DMA group
WGRP = 8               # lanes per weight DMA chunk / unpack group

PRECISION = "corrf8"

_cache = {}


def _build_corr():
    """Correction-filter kernel: c = g*x with packed circulant weights.

    All streams fp8e4m3. Per lane two DoubleRow matmuls (contraction 256
    = 2 k-tiles of 128, true 2x fp8 rate). HW requires the two k-tile
    blocks of the moving operand to be exactly adjacent (stride == block
    width), so x is laid out per lane as 768 chunk-columns

        [O_0..O_255 | E_0..E_255 | 0 O_0..O_254]

    (E/O = even/odd 128-chunks; odd chunks stored twice). The odd-output
    matmul reads cols [0,512) = (O_m cur, E_m prev), the even-output
    matmul reads cols [256,768) = (E_m cur, ZO_m prev); both use the SAME
    stationary [u0, u1]. Even/odd results land in separate PSUM banks
    (start_tensor_calc zeroes a whole 2 KiB bank) and one strided copy
    per lane casts both to fp8.
    """
    import concourse.tile as tile
    from concourse import bacc, mybir
    from concourse.ap import AP

    F32 = mybir.dt.float32
    F8 = mybir.dt.float8e4
    DR = mybir.MatmulPerfMode.DoubleRow
    nc = bacc.Bacc("TRN2", target_bir_lowering=False, debug=False)

    NM = NCH // 2  # double-chunks (moving columns per matmul)
    XC = 3 * NM    # x columns per lane
    # Per-core DRAM layouts (lane-minor so per-partition rows are contiguous):
    #   xq: [Q, LPC, XC]      x chunk columns [O | E | 0,O_0..254]
    #   wc: [Q, LPC, Q]       packed circulant Uc per lane
    #   c:  [Q, LPC, 2, NM]   c[j, l, e, m] = c_l[128*(2m+e) + 127 - j]
    xq_d = nc.dram_tensor("xq", [Q, LPC, XC], F8, kind="ExternalInput")
    wc_d = nc.dram_tensor("wc", [Q, LPC, Q], F8, kind="ExternalInput")
    c_d = nc.dram_tensor("c", [Q, LPC, 2, NM], F8, kind="ExternalOutput")

    NW = LPC // WGRP
    with tile.TileContext(nc) as tc:
        with ExitStack() as ctx:
            wpool = ctx.enter_context(tc.tile_pool(name="w", bufs=1))
            upool = ctx.enter_context(tc.tile_pool(name="u", bufs=1))
            xpool = ctx.enter_context(tc.tile_pool(name="x", bufs=2))
            ypool = ctx.enter_context(tc.tile_pool(name="y", bufs=3))
            pspool = ctx.enter_context(
                tc.tile_pool(name="ps", bufs=4, space="PSUM")
            )

            # wc group 0 first on the SP ring (small, unblocks the unpack);
            # remaining groups on the ACT ring, idle until copies start.
            wc_sb = []
            for k in range(NW):
                wt = wpool.tile([Q, WGRP, Q], F8, tag=f"wc{k}", name=f"wc{k}")
                wc_sb.append(wt)
                eng = nc.sync if k == 0 else nc.scalar
                eng.dma_start(wt[:], wc_d.ap()[:, k * WGRP : (k + 1) * WGRP, :])

            # Unpack complementary triangles with two independent
            # affine_selects per group (both read only wc; is_ge keeps
            # q+j<=127 -> u0, is_lt keeps the rest -> u1). GPSIMD-only op.
            # Group 0 split in halves so lane 0's stationary is ready sooner.
            u_sb = []
            for k in range(NW):
                u01 = upool.tile(
                    [Q, WGRP, 2, Q], F8, tag=f"u{k}", name=f"u{k}"
                )
                u_sb.append(u01)
                halves = (
                    [(0, WGRP // 2), (WGRP // 2, WGRP)] if k == 0
                    else [(0, WGRP)]
                )
                for lo, hi in halves:
                    # only is_ge is implemented in codegen, so u1 uses the
                    # negated affine (q+j-128 >= 0) instead of is_lt
                    for slot, base, cm, jst in (
                        (0, 127, -1, -1),
                        (1, -128, 1, 1),
                    ):
                        nc.gpsimd.affine_select(
                            u01[:, lo:hi, slot, :], wc_sb[k][:, lo:hi, :],
                            pattern=[[0, hi - lo], [jst, Q]],
                            compare_op=mybir.AluOpType.is_ge,
                            fill=0.0, base=base, channel_multiplier=cm,
                        )

            SGRP = 8  # lanes per store group
            for gx in range(LPC // XGRP):
                xgsl = slice(gx * XGRP, (gx + 1) * XGRP)
                xt = xpool.tile([Q, XGRP, XC], F8, tag="xq", name="xq_t")
                if gx == 0:
                    # lane 0 fine-grained so the first matmul's dependency
                    # is tiny; the rest of the group as one big transfer
                    nc.sync.dma_start(xt[:, 0:1, :], xq_d.ap()[:, 0:1, :])
                    nc.sync.dma_start(
                        xt[:, 1:XGRP, :], xq_d.ap()[:, 1:XGRP, :]
                    )
                else:
                    nc.sync.dma_start(xt[:], xq_d.ap()[:, xgsl, :])
                for g in range(gx * XGRP // SGRP, (gx + 1) * XGRP // SGRP):
                    gsl = slice(g * SGRP, (g + 1) * SGRP)
                    ct = ypool.tile([Q, SGRP, 2, NM], F8, tag="c", name="c_t")
                    for j in range(SGRP):
                        lane = g * SGRP + j
                        jx = lane - gx * XGRP
                        wk, wl = lane // WGRP, lane % WGRP
                        u01 = u_sb[wk]
                        ps = pspool.tile([Q, 2, NCH], F32, tag="ps", name="ps_t")
                        base = xt[:, jx, :]
                        pstr = list(base.ap[0])
                        xodd = AP(base.tensor, base.offset,
                                  [pstr, [NM, 2], [1, NM]])
                        xeven = AP(base.tensor, base.offset + NM,
                                   [pstr, [NM, 2], [1, NM]])
                        nc.tensor.matmul(
                            ps[:, 1, 0:NM], u01[:, wl, :, :], xodd,
                            start=True, stop=True, perf_mode=DR,
                        )
                        nc.tensor.matmul(
                            ps[:, 0, 0:NM], u01[:, wl, :, :], xeven,
                            start=True, stop=True, perf_mode=DR,
                        )
                        # PSUM->SBUF fp8 casts alternate ACT/DVE (GPSIMD
                        # cannot read PSUM) so neither engine exceeds the
                        # DMA stream time
                        if j % 2 == 0:
                            nc.scalar.copy(ct[:, j, :, :], ps[:, :, 0:NM])
                        else:
                            nc.vector.tensor_copy(ct[:, j, :, :], ps[:, :, 0:NM])
                    if g == LPC // SGRP - 1:
                        # halve the final store so the drain tail is shorter
                        h2 = SGRP // 2
                        nc.sync.dma_start(
                            c_d.ap()[:, g * SGRP : g * SGRP + h2, :, :],
                            ct[:, 0:h2, :, :],
                        )
                        nc.sync.dma_start(
                            c_d.ap()[:, g * SGRP + h2 : (g + 1) * SGRP, :, :],
                            ct[:, h2:SGRP, :, :],
                        )
                    else:
                        nc.sync.dma_start(c_d.ap()[:, gsl, :, :], ct[:])

    nc.compile()
    return nc


def _build_fp16pure():
    """Legacy full-filter fp16 kernel (x fp16, w fp16, y fp16; 256 taps)."""
    import concourse.tile as tile
    from concourse import bacc, mybir

    F32 = mybir.dt.float32
    F16 = mybir.dt.float16
    nc = bacc.Bacc("TRN2", target_bir_lowering=False, debug=False)

    xh_d = nc.dram_tensor("xh", [Q, LPC, NCH], F16, kind="ExternalInput")
    w_d = {
        n: nc.dram_tensor(n, [Q, LPC, Q], F16, kind="ExternalInput")
        for n in ("w0h", "w1h")
    }
    y_d = nc.dram_tensor("yt", [Q, LPC, NCH], F16, kind="ExternalOutput")

    with tile.TileContext(nc) as tc:
        with ExitStack() as ctx:
            wpool = ctx.enter_context(tc.tile_pool(name="w", bufs=1))
            xpool = ctx.enter_context(tc.tile_pool(name="x", bufs=4))
            ypool = ctx.enter_context(tc.tile_pool(name="y", bufs=6))
            pspool = ctx.enter_context(
                tc.tile_pool(name="ps", bufs=8, space="PSUM")
            )

            wbounds = [0, 1, WGRP] + list(range(2 * WGRP, LPC + 1, WGRP))
            w_sb = {}
            for n in w_d:
                w_sb[n] = [
                    wpool.tile(
                        [Q, wbounds[k + 1] - wbounds[k], Q], F16,
                        tag=f"{n}_{k}", name=f"{n}_{k}",
                    )
                    for k in range(len(wbounds) - 1)
                ]
            for k in range(len(wbounds) - 1):
                sl = slice(wbounds[k], wbounds[k + 1])
                for n in w_d:
                    nc.scalar.dma_start(w_sb[n][k][:], w_d[n].ap()[:, sl, :])

            for gx in range(LPC // XGRP):
                xgsl = slice(gx * XGRP, (gx + 1) * XGRP)
                xh = xpool.tile([Q, XGRP, NCH], F16, tag="xh", name="xh_t")
                if gx == 0:
                    nc.sync.dma_start(xh[:, 0:1, :], xh_d.ap()[:, 0:1, :])
                    nc.sync.dma_start(xh[:, 1:XGRP, :], xh_d.ap()[:, 1:XGRP, :])
                else:
                    nc.sync.dma_start(xh[:], xh_d.ap()[:, xgsl, :])
                for g in range(gx * XGRP // GRP, (gx + 1) * XGRP // GRP):
                    gsl = slice(g * GRP, (g + 1) * GRP)
                    yt = ypool.tile([Q, GRP, NCH], F16, tag="y", name="y_t")
                    for j in range(GRP):
                        lane = g * GRP + j
                        jx = lane - gx * XGRP
                        wk = next(
                            kk for kk in range(len(wbounds) - 1)
                            if lane < wbounds[kk + 1]
                        )
                        wl = lane - wbounds[wk]
                        ps = pspool.tile([Q, NCH], F32, tag="ps", name="ps_t")
                        mm = nc.tensor.matmul
                        mm(ps[:, :], w_sb["w0h"][wk][:, wl, :], xh[:, jx, :],
                           start=True, stop=False)
                        mm(ps[:, 1:NCH], w_sb["w1h"][wk][:, wl, :],
                           xh[:, jx, 0 : NCH - 1], start=False, stop=True)
                        if j % 2 == 0:
                            nc.scalar.copy(yt[:, j, :], ps[:, :])
                        else:
                            nc.vector.tensor_copy(yt[:, j, :], ps[:, :])
                    if g == LPC // GRP - 1:
                        h2 = GRP // 2
                        nc.scalar.dma_start(
                            y_d.ap()[:, g * GRP : g * GRP + h2, :], yt[:, 0:h2, :]
                        )
                        nc.scalar.dma_start(
                            y_d.ap()[:, g * GRP + h2 : (g + 1) * GRP, :],
                            yt[:, h2:GRP, :],
                        )
                    else:
                        nc.scalar.dma_start(y_d.ap()[:, gsl, :], yt[:])

    nc.compile()
    return nc


def _get_bass():
    key = ("nc", PRECISION)
    if key not in _cache:
        _cache[key] = (
            _build_corr() if PRECISION == "corrf8" else _build_fp16pure()
        )
    return _cache[key]


def _impulse_response(a: np.ndarray, ktaps: int) -> np.ndarray:
    """h[l, n] for n in [0, ktaps), float64 recurrence."""
    an = (a.astype(np.float64) / a[..., 0:1].astype(np.float64)).reshape(L, 17)
    h = np.zeros((L, ktaps), np.float64)
    h[:, 0] = 1.0
    for n in range(1, ktaps):
        k = np.arange(1, min(n, 16) + 1)
        h[:, n] = -np.einsum("lk,lk->l", an[:, k], h[:, n - k])
    return h


def _run(in_maps):
    from concourse import bass_utils

    nc = _get_bass()
    res = bass_utils.run_bass_kernel_spmd(
        nc,
        in_maps,
        core_ids=list(range(NCORES)),
        trace=bool(_cache.get("trace", False)),
        trace_cores=_cache.get("trace_cores"),
    )
    _cache["last_results"] = res
    return res


def _kernel_corr(x: np.ndarray) -> np.ndarray:
    import ml_dtypes

    F8 = ml_dtypes.float8_e4m3

    a = _cache.pop("a_pending")
    g = _impulse_response(a, KTAPS).astype(np.float32)
    g[:, 0] = 0.0  # tap 0 handled exactly on host (y = x + c)
    qi = np.arange(Q)
    idx = (127 - np.add.outer(qi, qi)) % 128  # Uc[q,j] = g[(127-q-j) mod 128]
    wc_all = np.ascontiguousarray(g[:, idx])  # [L, q, j] fp32

    NM = NCH // 2
    XC = 3 * NM
    x3 = x.reshape(L, NCH, Q)
    E = x3[:, 0::2, :].transpose(2, 0, 1).astype(F8)  # [q, l, m]
    O = x3[:, 1::2, :].transpose(2, 0, 1).astype(F8)
    xq = np.zeros((Q, L, XC), F8)  # col 2*NM stays zero (chunk -1)
    xq[:, :, 0:NM] = O
    xq[:, :, NM : 2 * NM] = E
    xq[:, :, 2 * NM + 1 :] = O[:, :, 0 : NM - 1]
    wc8 = wc_all.astype(F8)

    in_maps = []
    for core in range(NCORES):
        sl = slice(core * LPC, (core + 1) * LPC)
        in_maps.append(
            {
                "xq": np.ascontiguousarray(xq[:, sl, :]),
                "wc": np.ascontiguousarray(wc8[sl].transpose(1, 0, 2)),
            }
        )
    res = _run(in_maps)

    y = np.empty((L, T), np.float32)
    for core in range(NCORES):
        c = res.results[core]["c"].astype(np.float32)  # [j, lane, e, m]
        sl = slice(core * LPC, (core + 1) * LPC)
        # c[j, l, e, m] = c_l[128*(2m+e) + 127 - j]: unflip j, reinterleave
        y[sl] = c[::-1].transpose(1, 3, 2, 0).reshape(LPC, T)
    y += x.reshape(L, T)
    return y.reshape(B, C, T)


def _kernel_fp16pure(x: np.ndarray) -> np.ndarray:
    a = _cache.pop("a_pending")
    h = _impulse_response(a, 256).astype(np.float32)  # [L, 256]
    qi = np.arange(Q)
    d = qi[None, :] - qi[:, None]  # d[q, i] = i - q
    w0 = np.where(d >= 0, h[:, np.clip(d, 0, 255)], 0.0).astype(np.float32)
    w1 = h[:, d + Q].astype(np.float32)  # [L, q, i]

    xq = np.ascontiguousarray(
        x.reshape(L, NCH, Q).transpose(2, 0, 1)
    ).astype(np.float16)
    wmats = {"w0h": w0.astype(np.float16), "w1h": w1.astype(np.float16)}

    in_maps = []
    for core in range(NCORES):
        sl = slice(core * LPC, (core + 1) * LPC)
        m = {"xh": np.ascontiguousarray(xq[:, sl, :])}
        for n, w in wmats.items():
            m[n] = np.ascontiguousarray(w[sl].transpose(1, 0, 2))
        in_maps.append(m)
    res = _run(in_maps)

    y = np.empty((L, T), np.float32)
    for core in range(NCORES):
        yt = res.results[core]["yt"].astype(np.float32)  # [i, lane, c]
        sl = slice(core * LPC, (core + 1) * LPC)
        y[sl] = yt.transpose(1, 2, 0).reshape(LPC, T)
    return y.reshape(B, C, T)


def kernel(x: np.ndarray, a: np.ndarray) -> np.ndarray:
    x = np.ascontiguousarray(x, dtype=np.float32)
    a = np.ascontiguousarray(a, dtype=np.float32)
    _cache["a_pending"] = a
    if PRECISION == "corrf8":
        return _kernel_corr(x)
    return _kernel_fp16pure(x)
